# revision 7
# baseline (speedup 1.0000x reference)
"""TRN2 kernel for HAKMEM entangled complex attention — full on-device compute.

Head-parallel over 8 NeuronCores (2 heads/core). Host folds entanglement +
rope-pair de-interleave (PERM) + magnitude scale into projection weights;
phase shift applied doubled on K only (exact: score is a non-conjugated
complex bilinear form). On device per core: 6 projections in qT orientation
(dims on partitions), rope via DMA partition-shuffle + full-lane DVE ops,
phase via tensor_scalar, scores computed transposed ([k,q]) with stacked
[qr;qi]/[kr;-ki] operands (one 128-contraction matmul per score component),
flash-style causal block attention with exp(alpha*sqrt(ar^2+ai^2+1e-6)),
PV + ones-matmul row sums accumulated in PSUM, per-head normalization, then
an AllGather of per-head outputs and a column-parallel output projection
with zero-padded lhsT. Host only transposes/assembles I/O.
"""
import sys
sys.path.insert(0, "/opt/trn_rl_repo")
import numpy as np
import ml_dtypes

BF = ml_dtypes.bfloat16
DIM, H, Dh, ROT, S = 1024, 16, 64, 32, 2048
NCHUNK = DIM // 128        # 8 contraction chunks
NSEQT = S // 128           # 16 seq tiles
NQC = S // 512             # 4 q chunks
PERM = np.concatenate([np.arange(0, ROT, 2), np.arange(1, ROT, 2),
                       np.arange(ROT, Dh)])

_NC = None
_NC_KEY = None


def _build_nc(alpha):
    import concourse.tile as tile
    from concourse import bacc, mybir
    F32, BF16 = mybir.dt.float32, mybir.dt.bfloat16
    AF = mybir.ActivationFunctionType
    OP = mybir.AluOpType

    nc = bacc.Bacc("TRN2", target_bir_lowering=False, debug=False,
                   num_devices=8)
    # ---- DRAM I/O (per core)
    xTr = nc.dram_tensor("xTr", [DIM, S], BF16, kind="ExternalInput").ap()
    xTi = nc.dram_tensor("xTi", [DIM, S], BF16, kind="ExternalInput").ap()
    wqr = nc.dram_tensor("wqr", [DIM, 128], BF16, kind="ExternalInput").ap()
    wqi = nc.dram_tensor("wqi", [DIM, 128], BF16, kind="ExternalInput").ap()
    wkr = nc.dram_tensor("wkr", [DIM, 128], BF16, kind="ExternalInput").ap()
    wki = nc.dram_tensor("wki", [DIM, 128], BF16, kind="ExternalInput").ap()
    wvr = nc.dram_tensor("wvr", [DIM, 128], BF16, kind="ExternalInput").ap()
    wvi = nc.dram_tensor("wvi", [DIM, 128], BF16, kind="ExternalInput").ap()
    woR = nc.dram_tensor("woR", [2 * DIM, 128], BF16, kind="ExternalInput").ap()
    woI = nc.dram_tensor("woI", [2 * DIM, 128], BF16, kind="ExternalInput").ap()
    smalls = nc.dram_tensor("smalls", [128, 13], F32, kind="ExternalInput").ap()
    coefA = nc.dram_tensor("coefA", [128, S], BF16, kind="ExternalInput").ap()
    coefB = nc.dram_tensor("coefB", [128, S], BF16, kind="ExternalInput").ap()
    masks = nc.dram_tensor("masks", [128, 4 * 512], BF16,
                           kind="ExternalInput").ap()
    onesd = nc.dram_tensor("onesd", [128, 1], BF16, kind="ExternalInput").ap()
    onesb = nc.dram_tensor("onesb", [1, 128], BF16, kind="ExternalInput").ap()
    outTr = nc.dram_tensor("outTr", [128, S], F32, kind="ExternalOutput").ap()
    outTi = nc.dram_tensor("outTi", [128, S], F32, kind="ExternalOutput").ap()

    # smalls column indices
    C_BQR, C_BQI, C_BKR, C_BKI, C_BVR, C_BVI = 0, 1, 2, 3, 4, 5
    C_C2, C_S2, C_NS2, C_BOR, C_BOI, C_EPS, C_Z = 6, 7, 8, 9, 10, 11, 12

    with tile.TileContext(nc) as tc:
        with tc.tile_pool(name="sb", bufs=1) as sb, \
             tc.tile_pool(name="tmp", bufs=2) as tp, \
             tc.tile_pool(name="dr", bufs=1, space="DRAM") as dp, \
             tc.tile_pool(name="ps", bufs=1, space="PSUM") as ps:

            cc_in = dp.tile([256, S], BF16, name="cc_in")
            cc_out = dp.tile([2048, S], BF16, name="cc_out",
                             addr_space="Shared")

            # ---------- load constants / weights / x
            sm = sb.tile([128, 13], F32, name="sm")
            nc.sync.dma_start(sm[:], smalls[:])
            cA = sb.tile([128, S], BF16, name="cA", tag="cf", bufs=2)
            cB = sb.tile([128, S], BF16, name="cB", tag="cf", bufs=2)
            nc.sync.dma_start(cA[:], coefA[:])
            nc.sync.dma_start(cB[:], coefB[:])
            msk = sb.tile([128, 4 * 512], BF16, name="msk")
            nc.sync.dma_start(msk[:], masks[:])
            od = sb.tile([128, 1], BF16, name="od")
            nc.sync.dma_start(od[:], onesd[:])
            ob = sb.tile([1, 128], BF16, name="ob")
            nc.sync.dma_start(ob[:], onesb[:])

            wt = {}
            for nm, dr in (("wqr", wqr), ("wqi", wqi), ("wkr", wkr),
                           ("wki", wki), ("wvr", wvr), ("wvi", wvi)):
                t = sb.tile([128, DIM], BF16, name=f"wt_{nm}")
                for k in range(NCHUNK):
                    nc.sync.dma_start(t[:, k * 128:(k + 1) * 128],
                                      dr[k * 128:(k + 1) * 128, :])
                wt[nm] = t
            wo = {}
            for nm, dr in (("woR", woR), ("woI", woI)):
                t = sb.tile([128, 2 * DIM], BF16, name=f"wo_{nm}")
                for k in range(2 * NCHUNK):
                    nc.sync.dma_start(t[:, k * 128:(k + 1) * 128],
                                      dr[k * 128:(k + 1) * 128, :])
                wo[nm] = t

            xt = []  # 16 tiles: xTr chunks 0..7, then xTi chunks 0..7
            for s_i, dr in ((0, xTr), (1, xTi)):
                for k in range(NCHUNK):
                    t = sb.tile([128, S], BF16, name=f"xt{s_i}_{k}",
                                tag="xt", bufs=16)
                    nc.sync.dma_start(t[:], dr[k * 128:(k + 1) * 128, :])
                    xt.append(t)

            # ---------- projections (qT orientation) -> stage tiles
            # stage row layout per series: [h0: x1(16) x2(16) xp(32) | h1: ...]
            stages = {}
            for nm, w, xoff, bcol in (("qr", "wqr", 0, C_BQR),
                                      ("qi", "wqi", 8, C_BQI),
                                      ("kr", "wkr", 0, C_BKR),
                                      ("ki", "wki", 8, C_BKI),
                                      ("vr", "wvr", 0, C_BVR),
                                      ("vi", "wvi", 8, C_BVI)):
                st = sb.tile([128, S], BF16, name=f"st_{nm}", tag="stg", bufs=8)
                for j in range(NQC):
                    pp = ps.tile([128, 512], F32, name="pp", tag="pp", bufs=2)
                    for k in range(NCHUNK):
                        nc.tensor.matmul(
                            pp[:], wt[w][:, k * 128:(k + 1) * 128],
                            xt[xoff + k][:, j * 512:(j + 1) * 512],
                            start=(k == 0), stop=(k == NCHUNK - 1))
                    nc.scalar.activation(st[:, j * 512:(j + 1) * 512], pp[:],
                                         AF.Identity,
                                         bias=sm[:, bcol:bcol + 1])
                stages[nm] = st

            # ---------- V: vT -> Vcomb via DMA transpose
            vc = []
            for hh in range(2):
                v = sb.tile([128, S], BF16, name=f"vc{hh}")
                for t_i in range(NSEQT):
                    nc.sync.dma_start(
                        v[:, t_i * 128 + 0:t_i * 128 + 64],
                        stages["vr"][hh * 64:(hh + 1) * 64,
                                     t_i * 128:(t_i + 1) * 128],
                        transpose=True)
                    nc.sync.dma_start(
                        v[:, t_i * 128 + 64:t_i * 128 + 128],
                        stages["vi"][hh * 64:(hh + 1) * 64,
                                     t_i * 128:(t_i + 1) * 128],
                        transpose=True)
                vc.append(v)

            # ---------- rope (DMA 16-row shuffles + full-lane DVE)
            roped = {}
            for nm in ("qr", "qi", "kr", "ki"):
                st = stages[nm]
                sh = sb.tile([128, S], BF16, name=f"sh_{nm}", tag="shuf", bufs=2)
                for b in (0, 64):
                    nc.sync.dma_start(sh[b + 16:b + 32, :], st[b:b + 16, :])
                    nc.sync.dma_start(sh[b:b + 16, :], st[b + 16:b + 32, :])
                    nc.sync.dma_start(sh[b + 32:b + 64, :],
                                      st[b + 32:b + 64, :])
                ro = sb.tile([128, S], BF16, name=f"ro_{nm}", tag="stg", bufs=8)
                t1 = tp.tile([128, S], BF16, name="t1", tag="ropet")
                t2 = tp.tile([128, S], BF16, name="t2", tag="ropet")
                nc.vector.tensor_mul(t1[:], st[:], cA[:])
                nc.vector.tensor_mul(t2[:], sh[:], cB[:])
                nc.vector.tensor_add(ro[:], t1[:], t2[:])
                roped[nm] = ro

            # ---------- phase (doubled) on K, build per-head stacks
            krR, kiR = roped["kr"], roped["ki"]
            kr2 = sb.tile([128, S], BF16, name="kr2", tag="stg", bufs=8)
            ki2 = sb.tile([128, S], BF16, name="ki2", tag="stg", bufs=8)
            nki2 = sb.tile([128, S], BF16, name="nki2", tag="stg", bufs=8)
            u1 = tp.tile([128, S], BF16, name="u1", tag="ropet")
            nc.vector.tensor_scalar_mul(u1[:], krR[:], sm[:, C_C2:C_C2 + 1])
            nc.vector.scalar_tensor_tensor(kr2[:], kiR[:],
                                           sm[:, C_NS2:C_NS2 + 1], u1[:],
                                           OP.mult, OP.add)
            u2 = tp.tile([128, S], BF16, name="u2", tag="ropet")
            nc.vector.tensor_scalar_mul(u2[:], krR[:], sm[:, C_S2:C_S2 + 1])
            nc.vector.scalar_tensor_tensor(ki2[:], kiR[:],
                                           sm[:, C_C2:C_C2 + 1], u2[:],
                                           OP.mult, OP.add)
            nc.vector.tensor_scalar_mul(nki2[:], ki2[:], -1.0)

            # per-head stacks via DMA partition moves
            QS, KA, KB = [], [], []
            for hh in range(2):
                q = sb.tile([128, S], BF16, name=f"QS{hh}")
                a = sb.tile([128, S], BF16, name=f"KA{hh}")
                b = sb.tile([128, S], BF16, name=f"KB{hh}")
                r = slice(hh * 64, (hh + 1) * 64)
                nc.sync.dma_start(q[0:64, :], roped["qr"][r, :])
                nc.sync.dma_start(q[64:128, :], roped["qi"][r, :])
                nc.sync.dma_start(a[0:64, :], kr2[r, :])
                nc.sync.dma_start(a[64:128, :], nki2[r, :])
                nc.sync.dma_start(b[0:64, :], ki2[r, :])
                nc.sync.dma_start(b[64:128, :], kr2[r, :])
                QS.append(q); KA.append(a); KB.append(b)

            # ---------- attention (transposed scores, causal block skipping)
            yT = []
            for hh in range(2):
                y = sb.tile([128, S], BF16, name=f"yT{hh}")
                for j in range(NQC):
                    qs = slice(j * 512, (j + 1) * 512)
                    py = ps.tile([128, 512], F32, name="py", tag="py")
                    pd = ps.tile([1, 512], F32, name="pd", tag="pd")
                    nkt = 4 * j + 4
                    for t_i in range(nkt):
                        ks = slice(t_i * 128, (t_i + 1) * 128)
                        par = ps.tile([128, 512], F32, name="par", tag="par",
                                      bufs=2)
                        pai = ps.tile([128, 512], F32, name="pai", tag="pai",
                                      bufs=2)
                        nc.tensor.matmul(par[:], KA[hh][:, ks], QS[hh][:, qs],
                                         start=True, stop=True)
                        nc.tensor.matmul(pai[:], KB[hh][:, ks], QS[hh][:, qs],
                                         start=True, stop=True)
                        t1 = tp.tile([128, 512], BF16, name="sq1", tag="sq1")
                        cp = tp.tile([128, 512], BF16, name="cp", tag="cp")
                        t2 = tp.tile([128, 512], BF16, name="sq2", tag="sq2")
                        nc.scalar.activation(t1[:], par[:], AF.Square,
                                             bias=sm[:, C_Z:C_Z + 1])
                        nc.vector.tensor_copy(cp[:], pai[:])
                        nc.vector.tensor_tensor(t2[:], cp[:], cp[:], OP.mult)
                        ss = tp.tile([128, 512], BF16, name="ss", tag="ss")
                        nc.vector.tensor_add(ss[:], t1[:], t2[:])
                        mg = tp.tile([128, 512], BF16, name="mg", tag="mg")
                        nc.scalar.activation(mg[:], ss[:], AF.Sqrt,
                                             bias=sm[:, C_EPS:C_EPS + 1])
                        pte = tp.tile([128, 512], BF16, name="pte", tag="pte",
                                      bufs=2)
                        nc.scalar.activation(pte[:], mg[:], AF.Exp,
                                             bias=sm[:, C_Z:C_Z + 1],
                                             scale=float(alpha))
                        m = t_i - 4 * j
                        if m >= 0:
                            pt = tp.tile([128, 512], BF16, name="pt",
                                         tag="pt", bufs=2)
                            nc.vector.tensor_mul(
                                pt[:], pte[:], msk[:, m * 512:(m + 1) * 512])
                        else:
                            pt = pte
                        nc.tensor.matmul(py[:], vc[hh][:, ks], pt[:],
                                         start=(t_i == 0),
                                         stop=(t_i == nkt - 1))
                        nc.tensor.matmul(pd[:], od[:], pt[:],
                                         start=(t_i == 0),
                                         stop=(t_i == nkt - 1))
                    # normalize: broadcast 1/d via ones-matmul + reciprocal
                    dsb = tp.tile([1, 512], BF16, name="dsb", tag="dsb")
                    nc.scalar.activation(dsb[:], pd[:], AF.Copy)
                    pbc = ps.tile([128, 512], F32, name="pbc", tag="pp",
                                  bufs=2)
                    nc.tensor.matmul(pbc[:], ob[:], dsb[:],
                                     start=True, stop=True)
                    rbc = tp.tile([128, 512], F32, name="rbc", tag="cf", bufs=2)
                    nc.vector.reciprocal(rbc[:], pbc[:])
                    nc.vector.tensor_mul(y[:, qs], py[:], rbc[:])
                yT.append(y)

            # ---------- AllGather of per-head outputs
            nc.sync.dma_start(cc_in[0:128, :], yT[0][:])
            nc.sync.dma_start(cc_in[128:256, :], yT[1][:])
            nc.gpsimd.collective_compute(
                "AllGather", OP.bypass,
                replica_groups=[list(range(8))],
                ins=[cc_in[:]], outs=[cc_out[:]])

            # ---------- output projection (zero-padded interleaved lhsT)
            ag = []
            for k in range(2 * NCHUNK):
                t = sb.tile([128, S], BF16, name=f"ag{k}", tag="xt", bufs=16)
                nc.sync.dma_start(t[:], cc_out[k * 128:(k + 1) * 128, :])
                ag.append(t)
            for nm, out_dr, bcol in (("woR", outTr, C_BOR),
                                     ("woI", outTi, C_BOI)):
                for j in range(NQC):
                    po = ps.tile([128, 512], F32, name="po", tag="py")
                    for k in range(2 * NCHUNK):
                        nc.tensor.matmul(po[:],
                                         wo[nm][:, k * 128:(k + 1) * 128],
                                         ag[k][:, j * 512:(j + 1) * 512],
                                         start=(k == 0),
                                         stop=(k == 2 * NCHUNK - 1))
                    osb = tp.tile([128, 512], F32, name="osb", tag="cf", bufs=2)
                    nc.scalar.activation(osb[:], po[:], AF.Identity,
                                         bias=sm[:, bcol:bcol + 1])
                    nc.sync.dma_start(out_dr[:, j * 512:(j + 1) * 512],
                                      osb[:])
    nc.compile()
    return nc


def _prep(inputs):
    E = np.asarray(inputs["entanglement_matrix"], np.float32)

    def fold(W, b, scale=1.0):
        W4 = np.asarray(W, np.float32).reshape(H, Dh, DIM)[:, PERM, :]
        b4 = np.asarray(b, np.float32).reshape(H, Dh)[:, PERM]
        W4 = np.einsum("hx,hdD->xdD", E, W4) * scale
        b4 = np.einsum("hx,hd->xd", E, b4) * scale
        return W4.reshape(DIM, DIM), b4.reshape(DIM)

    eps = 1 / (1 + np.exp(-float(inputs["circle_epsilon"]))) * 0.03
    magsc = np.sqrt((1 + eps * eps) / Dh)
    temp = max(np.exp(float(inputs["attention_temperature"])), 0.1)
    alpha = 1 / (1 + np.exp(-float(inputs["interference_strength"]))) / temp

    Wq_r, bq_r = fold(inputs["Wq_r"], inputs["bq_r"])
    Wq_i, bq_i = fold(inputs["Wq_i"], inputs["bq_i"])
    Wk_r, bk_r = fold(inputs["Wk_r"], inputs["bk_r"], magsc)
    Wk_i, bk_i = fold(inputs["Wk_i"], inputs["bk_i"], magsc)
    Wv_r = np.asarray(inputs["Wv_r"], np.float32)
    Wv_i = np.asarray(inputs["Wv_i"], np.float32)
    bv_r = np.asarray(inputs["bv_r"], np.float32)
    bv_i = np.asarray(inputs["bv_i"], np.float32)
    Wo_r = np.asarray(inputs["Wo_r"], np.float32)
    Wo_i = np.asarray(inputs["Wo_i"], np.float32)

    xTr = np.ascontiguousarray(
        np.asarray(inputs["real"], np.float32)[0].T).astype(BF)
    xTi = np.ascontiguousarray(
        np.asarray(inputs["imag"], np.float32)[0].T).astype(BF)

    pos = np.arange(S, dtype=np.float32)
    ang = np.outer(pos, np.asarray(inputs["rotary_freqs"], np.float32))
    c, s = np.cos(ang).T.astype(np.float32), np.sin(ang).T.astype(np.float32)
    # coefficient tiles: per 64-block rows [0:16]=A:c B:-s, [16:32]=A:c B:s,
    # [32:64]=A:1 B:0
    A = np.ones((128, S), np.float32)
    B = np.zeros((128, S), np.float32)
    for b in (0, 64):
        A[b:b + 16] = c; A[b + 16:b + 32] = c
        B[b:b + 16] = -s; B[b + 16:b + 32] = s
    coefA, coefB = A.astype(BF), B.astype(BF)

    ph = np.asarray(inputs["phase_shifts"], np.float32).reshape(H, Dh)[:, PERM]
    c2a, s2a = np.cos(2 * ph), np.sin(2 * ph)

    kk = np.arange(128)[:, None]
    qq = np.arange(512)[None, :]
    masks = np.concatenate(
        [(kk + 128 * m <= qq).astype(np.float32) for m in range(4)],
        axis=1).astype(BF)
    onesd = np.ones((128, 1), np.float32).astype(BF)
    onesb = np.ones((1, 128), np.float32).astype(BF)

    in_maps = []
    for cc in range(8):
        sl = slice(128 * cc, 128 * (cc + 1))
        h0, h1 = 2 * cc, 2 * cc + 1
        sm = np.zeros((128, 13), np.float32)
        sm[:, 11] = 1e-6
        sm[:, 0], sm[:, 1] = bq_r[sl], bq_i[sl]
        sm[:, 2], sm[:, 3] = bk_r[sl], bk_i[sl]
        sm[:, 4], sm[:, 5] = bv_r[sl], bv_i[sl]
        c2c = np.concatenate([c2a[h0], c2a[h1]])
        s2c = np.concatenate([s2a[h0], s2a[h1]])
        sm[:, 6], sm[:, 7], sm[:, 8] = c2c, s2c, -s2c
        sm[:, 9], sm[:, 10] = (np.asarray(inputs["bo_r"], np.float32)[sl],
                               np.asarray(inputs["bo_i"], np.float32)[sl])
        # zero-padded interleaved output-projection lhsT: AG row r holds
        # (core k, head parity p, half q): woR rows nonzero only at yr halves
        woRz = np.zeros((2 * DIM, 128), np.float32)
        woIz = np.zeros((2 * DIM, 128), np.float32)
        for h in range(16):
            base = 128 * h
            woRz[base:base + 64, :] = Wo_r[sl, 64 * h:64 * (h + 1)].T
            woIz[base + 64:base + 128, :] = Wo_i[sl, 64 * h:64 * (h + 1)].T
        in_maps.append({
            "xTr": xTr, "xTi": xTi,
            "wqr": np.ascontiguousarray(Wq_r[sl].T).astype(BF),
            "wqi": np.ascontiguousarray(Wq_i[sl].T).astype(BF),
            "wkr": np.ascontiguousarray(Wk_r[sl].T).astype(BF),
            "wki": np.ascontiguousarray(Wk_i[sl].T).astype(BF),
            "wvr": np.ascontiguousarray(Wv_r[sl].T).astype(BF),
            "wvi": np.ascontiguousarray(Wv_i[sl].T).astype(BF),
            "woR": woRz.astype(BF), "woI": woIz.astype(BF),
            "smalls": sm, "coefA": coefA, "coefB": coefB,
            "masks": masks, "onesd": onesd, "onesb": onesb,
        })
    return in_maps, alpha


def kernel(**inputs):
    global _NC, _NC_KEY
    in_maps, alpha = _prep(inputs)
    key = float(np.float32(alpha))
    if _NC is None or _NC_KEY != key:
        _NC = _build_nc(key)
        _NC_KEY = key
    from concourse.bass_utils import run_bass_kernel_spmd
    res = run_bass_kernel_spmd(_NC, in_maps, core_ids=list(range(8)))
    out_r = np.empty((S, DIM), np.float32)
    out_i = np.empty((S, DIM), np.float32)
    for cc in range(8):
        sl = slice(128 * cc, 128 * (cc + 1))
        out_r[:, sl] = res.results[cc]["outTr"].T
        out_i[:, sl] = res.results[cc]["outTi"].T
    return out_r[None], out_i[None]
